# revision 51
# baseline (speedup 1.0000x reference)
"""Trainium2 Bass kernel for nn_ChunkProcessor (segment-mean -> 2-layer
transformer encoder over chunks -> gather-expand -> final LN).

Sharding: data-parallel over batch B=8 across the 8 NeuronCores; each core
processes one batch item end to end (no cross-core communication).

Device pipeline per core:
  1. segment mean: one-hot(seg) matmuls on PE accumulate sums[C,D] and
     counts[C] in PSUM while token tiles stream in over HWDGE (cast to bf16
     on ACT/DVE). Because segment ids are sorted, each 128-token tile only
     touches a small contiguous range of chunk tiles; the SPMD program uses
     the union of those ranges across the 8 batch items (host-computed).
  2. transformer (bf16 matmuls, fp32 PSUM/LN): activations kept feature-major
     [d, c] for matmuls, token-major [c, d] for LayerNorm; softmax without
     max-subtraction (scores bounded ~1) so attention needs no big transposes.
  3. final LN on the [C, D] chunk table, then expand back to [S, D] with
     one-hot-transposed matmuls (same span-bound trick), streaming fp32
     tiles straight to HBM.
"""

import numpy as np

B, S, D = 8, 8192, 512
C, H, L, DFF = 512, 8, 2, 2048
HD = D // H          # 64
NT = S // 128        # 64 token tiles
CT = C // 128        # 4 chunk tiles
DT = D // 128        # 4 feature tiles
FT = DFF // 128      # 16
EPS = 1e-5

_CACHE = {}


def _build(flags, ranges):
    """Build the Bass program.

    flags  = (qkv_b, out_b, ff1_b, ff2_b, ln1_aff, ln2_aff, fln_aff) bools.
    ranges = tuple of (lo_m, hi_m) per token tile t: the contiguous range of
             chunk tiles any batch item's tile-t segment ids fall into.
    """
    import concourse.bass as bass
    import concourse.tile as tile
    from concourse import bacc, mybir
    from concourse.masks import make_identity

    (has_qkv_b, has_out_b, has_ff1_b, has_ff2_b,
     has_ln1, has_ln2, has_fln) = flags

    # first/last contributing token tile per chunk tile (for PSUM start/stop)
    first_t = [min(t for t in range(NT) if ranges[t][0] <= m <= ranges[t][1])
               for m in range(CT)]
    last_t = [max(t for t in range(NT) if ranges[t][0] <= m <= ranges[t][1])
              for m in range(CT)]

    f32 = mybir.dt.float32
    bf16 = mybir.dt.bfloat16
    f16 = mybir.dt.float16
    AL = mybir.AluOpType
    AF = mybir.ActivationFunctionType

    # Bacc (not raw Bass): its finalize pass legalizes multi-semaphore waits
    # into nop carriers — walrus TPB structs allow only ONE wait/instruction.
    nc = bacc.Bacc("TRN2", target_bir_lowering=False)

    tokens = nc.declare_dram_parameter("tokens", [S, D], bf16, isOutput=False)
    seg_col = nc.declare_dram_parameter("seg_col", [128, NT], f32, isOutput=False)
    seg_row = nc.declare_dram_parameter("seg_row", [1, S], f16, isOutput=False)
    iota_row = nc.declare_dram_parameter("iota_row", [128, C], f16, isOutput=False)
    iota_col = nc.declare_dram_parameter("iota_col", [128, CT], f32, isOutput=False)
    rcnt = nc.declare_dram_parameter("rcnt", [128, CT], f32, isOutput=False)
    wqkvT = nc.declare_dram_parameter("wqkvT", [L, D, 3 * D], bf16, isOutput=False)
    woT = nc.declare_dram_parameter("woT", [L, D, D], bf16, isOutput=False)
    w1T = nc.declare_dram_parameter("w1T", [L, D, DFF], bf16, isOutput=False)
    w2T = nc.declare_dram_parameter("w2T", [L, DFF, D], bf16, isOutput=False)
    if has_qkv_b:
        bqkv_c = nc.declare_dram_parameter("bqkv_c", [L, 128, 12], f32, isOutput=False)
        vb_row = nc.declare_dram_parameter("vb_row", [L, 1, D], f32, isOutput=False)
    if has_ff1_b:
        b1_c = nc.declare_dram_parameter("b1_c", [L, 128, FT], f32, isOutput=False)
    if has_out_b:
        outb_row = nc.declare_dram_parameter("outb_row", [L, 1, D], f32, isOutput=False)
    if has_ff2_b:
        ff2b_row = nc.declare_dram_parameter("ff2b_row", [L, 1, D], f32, isOutput=False)
    if has_ln1:
        ln1w_row = nc.declare_dram_parameter("ln1w_row", [L, 1, D], f32, isOutput=False)
        ln1b_row = nc.declare_dram_parameter("ln1b_row", [L, 1, D], f32, isOutput=False)
    if has_ln2:
        ln2w_row = nc.declare_dram_parameter("ln2w_row", [L, 1, D], f32, isOutput=False)
        ln2b_row = nc.declare_dram_parameter("ln2b_row", [L, 1, D], f32, isOutput=False)
    if has_fln:
        flnw_row = nc.declare_dram_parameter("flnw_row", [1, D], f32, isOutput=False)
        flnb_row = nc.declare_dram_parameter("flnb_row", [1, D], f32, isOutput=False)
    out_d = nc.declare_dram_parameter("out", [S, D], f32, isOutput=True)

    def bcast_load(pool, dram_row, tag):
        """DMA a [1, D] DRAM row into a [128, D] SBUF tile (partition bcast)."""
        t = pool.tile([128, D], f32, tag=tag, name=f"row_{tag}")
        src = bass.AP(tensor=dram_row.tensor, offset=dram_row.offset,
                      ap=[[0, 128]] + [list(p) for p in dram_row.ap[1:]])
        nc.gpsimd.dma_start(out=t, in_=src)
        return t

    with tile.TileContext(nc) as tc:
        with (
            tc.tile_pool(name="consts", bufs=1) as consts,
            tc.tile_pool(name="acts", bufs=1) as acts,
            tc.tile_pool(name="xm", bufs=2) as xmp,
            tc.tile_pool(name="xt", bufs=2) as xtp,
            tc.tile_pool(name="lnp", bufs=3) as lnp,
            tc.tile_pool(name="rows", bufs=1) as rows,
            tc.tile_pool(name="segbc", bufs=1) as segbc,
            tc.tile_pool(name="expq", bufs=4) as expq,
            tc.tile_pool(name="opp", bufs=2) as opp,
        ):
            # ---------------- constants ----------------
            seg_col_sb = consts.tile([128, NT], f32)
            nc.sync.dma_start(out=seg_col_sb, in_=seg_col[:, :])
            iota_row_sb = consts.tile([128, C], f16)
            nc.sync.dma_start(out=iota_row_sb, in_=iota_row[:, :])
            iota_col_sb = consts.tile([128, CT], f32)
            nc.sync.dma_start(out=iota_col_sb, in_=iota_col[:, :])
            rcnt_sb = consts.tile([128, CT], f32)
            nc.sync.dma_start(out=rcnt_sb, in_=rcnt[:, :])
            ident32 = consts.tile([128, 128], f32)
            make_identity(nc, ident32)
            ident_bf = consts.tile([128, 128], bf16)
            make_identity(nc, ident_bf)
            eps_t = consts.tile([128, 1], f32)
            nc.vector.memset(eps_t, EPS)

            # y_bf lives in the persistent pool (used by the expand phase
            # after the weight pool is closed).
            y_bf = acts.tile([128, CT, D], bf16, tag="y_bf")

            seg_bcs = []

            def ln_block(ps_src, resid, wrow, brow, out_ap):
                # out = LN(ps_src + resid) [* w + b]  (token-major)
                t_ = lnp.tile([128, D], f32, tag="ln_t", name="ln_t")
                if resid is not None:
                    nc.vector.tensor_tensor(out=t_, in0=ps_src, in1=resid, op=AL.add)
                else:
                    nc.vector.tensor_copy(t_, ps_src)
                st = lnp.tile([128, 6], f32, tag="ln_st", name="ln_st")
                nc.vector.bn_stats(out=st, in_=t_)
                mv = lnp.tile([128, 2], f32, tag="ln_mv", name="ln_mv")
                nc.vector.bn_aggr(out=mv, in_=st)
                sd = lnp.tile([128, 1], f32, tag="ln_sd", name="ln_sd")
                nc.scalar.activation(out=sd, in_=mv[:, 1:2], func=AF.Sqrt,
                                     bias=eps_t[:, 0:1], scale=1.0)
                rs = lnp.tile([128, 1], f32, tag="ln_rs", name="ln_rs")
                nc.vector.reciprocal(rs, sd)
                if wrow is None:
                    nc.vector.tensor_scalar(
                        out=out_ap, in0=t_, scalar1=mv[:, 0:1], scalar2=rs[:, 0:1],
                        op0=AL.subtract, op1=AL.mult)
                else:
                    xn = lnp.tile([128, D], f32, tag="ln_xn", name="ln_xn")
                    nc.vector.tensor_scalar(
                        out=xn, in0=t_, scalar1=mv[:, 0:1], scalar2=rs[:, 0:1],
                        op0=AL.subtract, op1=AL.mult)
                    nc.vector.tensor_tensor(out=xn, in0=xn, in1=wrow, op=AL.mult)
                    nc.vector.tensor_tensor(out=out_ap, in0=xn, in1=brow, op=AL.add)

            # ============ scope: weights + segsum + transformer ============
            with (
                tc.tile_pool(name="wts", bufs=1) as wts,
            ):
                # ---- weights (bf16, SWDGE) ----
                # Only layer 0's qkv/out weights load immediately; the rest
                # are gated behind the end of phase 1 (via a WAR dependency
                # on `gate`, which doubles as the last token tile's one-hot
                # buffer) so the token stream gets the HBM bandwidth first.
                wqkv_sb, wo_sb, w1_sb, w2_sb = [], [], [], []
                wq = wts.tile([128, DT, 3 * D], bf16, tag="wqkv0", name="wqkv0")
                nc.gpsimd.dma_start(
                    out=wq, in_=wqkvT[0].rearrange("(dt p) e -> p dt e", p=128))
                wqkv_sb.append(wq)
                wo = wts.tile([128, DT, D], bf16, tag="wo0", name="wo0")
                nc.gpsimd.dma_start(
                    out=wo, in_=woT[0].rearrange("(dt p) e -> p dt e", p=128))
                wo_sb.append(wo)

                # Late-weight tiles are allocated now but DMA'd only after the
                # last six token groups have landed IN these very buffers
                # (each tok4 group tg>=10 is staged in a slice of one late
                # tile) — the WAW/WAR dependency is the only thing the
                # scheduler respects, and it keeps these 8.6 MB of loads from
                # stealing HBM bandwidth from the token stream.
                w1_0 = wts.tile([128, DT, DFF], bf16, tag="w1l0", name="w1l0")
                w2_0 = wts.tile([128, FT, D], bf16, tag="w2l0", name="w2l0")
                wq_1 = wts.tile([128, DT, 3 * D], bf16, tag="wqkv1", name="wqkv1")
                wo_1 = wts.tile([128, DT, D], bf16, tag="wo1", name="wo1")
                w1_1 = wts.tile([128, DT, DFF], bf16, tag="w1l1", name="w1l1")
                w2_1 = wts.tile([128, FT, D], bf16, tag="w2l1", name="w2l1")
                w1_sb += [w1_0, w1_1]
                w2_sb += [w2_0, w2_1]
                wqkv_sb.append(wq_1)
                wo_sb.append(wo_1)
                tok_gates = {
                    10: w1_0[:, :, 0:512],
                    11: w2_0[:, 0:4, :],
                    12: wq_1[:, :, 0:512],
                    13: wo_1[:, :, :],
                    14: w1_1[:, :, 0:512],
                    15: w2_1[:, 0:4, :],
                }

                def load_late_weights():
                    nc.gpsimd.dma_start(
                        out=w1_0, in_=w1T[0].rearrange("(dt p) e -> p dt e", p=128))
                    nc.gpsimd.dma_start(
                        out=w2_0, in_=w2T[0].rearrange("(ft p) e -> p ft e", p=128))
                    nc.gpsimd.dma_start(
                        out=wq_1, in_=wqkvT[1].rearrange("(dt p) e -> p dt e", p=128))
                    nc.gpsimd.dma_start(
                        out=wo_1, in_=woT[1].rearrange("(dt p) e -> p dt e", p=128))
                    nc.gpsimd.dma_start(
                        out=w1_1, in_=w1T[1].rearrange("(dt p) e -> p dt e", p=128))
                    nc.gpsimd.dma_start(
                        out=w2_1, in_=w2T[1].rearrange("(ft p) e -> p ft e", p=128))
                    # prefetch the first few expand-phase segment-id
                    # broadcasts (tiny 1 KB reads, 128x SBUF fanout); the
                    # rest are loaded at the start of phase 3.
                    seg_row_ap = seg_row[:, :]
                    for g in range(6):
                        sb = segbc.tile([128, 512], f16, tag=f"seg_bc{g}",
                                        name=f"seg_bc{g}")
                        src = bass.AP(tensor=seg_row_ap.tensor, offset=g * 512,
                                      ap=[[0, 128], [1, 512]])
                        nc.gpsimd.dma_start(out=sb, in_=src)
                        seg_bcs.append(sb)

                bqkv_sb, b1_sb = [], []
                vb_sb, outb_sb, ff2b_sb = [], [], []
                ln1w_sb, ln1b_sb, ln2w_sb, ln2b_sb = [], [], [], []
                for l in range(L):
                    if has_qkv_b:
                        bq = consts.tile([128, 12], f32, tag=f"bqkv{l}", name=f"bqkv{l}")
                        nc.sync.dma_start(out=bq, in_=bqkv_c[l])
                        bqkv_sb.append(bq)
                        vb_sb.append(bcast_load(rows, vb_row[l], f"vb{l}"))
                    if has_ff1_b:
                        b1 = consts.tile([128, FT], f32, tag=f"b1{l}", name=f"b1{l}")
                        nc.sync.dma_start(out=b1, in_=b1_c[l])
                        b1_sb.append(b1)
                    if has_out_b:
                        outb_sb.append(bcast_load(rows, outb_row[l], f"outb{l}"))
                    if has_ff2_b:
                        ff2b_sb.append(bcast_load(rows, ff2b_row[l], f"ff2b{l}"))
                    if has_ln1:
                        ln1w_sb.append(bcast_load(rows, ln1w_row[l], f"ln1w{l}"))
                        ln1b_sb.append(bcast_load(rows, ln1b_row[l], f"ln1b{l}"))
                    if has_ln2:
                        ln2w_sb.append(bcast_load(rows, ln2w_row[l], f"ln2w{l}"))
                        ln2b_sb.append(bcast_load(rows, ln2b_row[l], f"ln2b{l}"))
                flnw_sb = bcast_load(rows, flnw_row, "flnw") if has_fln else None
                flnb_sb = bcast_load(rows, flnb_row, "flnb") if has_fln else None

                # ------------ phase 1: segment sums ------------
                # Layer-0 activations are built INSIDE phase 1 where data
                # permits: chunk tiles 0-1 close early (sorted segment ids),
                # so their divide + transpose + qkv-q/k matmuls are hoisted
                # into the PE idle time while the token tail streams in.
                x0 = xmp.tile([128, CT, D], f32, tag="xm", name="x0")
                xT0 = xtp.tile([128, DT, C], bf16, tag="xT", name="xT0")
                qT0 = acts.tile([128, DT, C], bf16, tag="qT", name="qT0")
                kT0 = acts.tile([128, DT, C], bf16, tag="kT", name="kT0")
                v_ext0 = acts.tile([128, CT, H, HD + 1], bf16, tag="v_ext",
                                   name="v_ext0")
                nc.vector.memset(v_ext0[:, :, :, HD:HD + 1], 1.0)
                with (
                    tc.tile_pool(name="pseg", bufs=1, space="PSUM") as pseg,
                    tc.tile_pool(name="psh", bufs=2, space="PSUM") as psh,
                    tc.tile_pool(name="toks", bufs=2) as toks,
                    tc.tile_pool(name="ohs", bufs=4) as ohs,
                ):
                    ps_sums = [pseg.tile([128, D], f32, tag=f"sums{m}", name=f"sums{m}")
                               for m in range(CT)]

                    def hoist_half(half):
                        cts = (0, 1) if half == 0 else (2, 3)
                        csl = slice(half * 256, half * 256 + 256)
                        for m in cts:
                            nc.vector.tensor_scalar(
                                out=x0[:, m, :], in0=ps_sums[m],
                                scalar1=rcnt_sb[:, m:m + 1],
                                scalar2=None, op0=AL.mult)
                        for i in cts:
                            for j in range(DT):
                                pst = psh.tile([128, 128], f32, tag="ps_t1",
                                               name="ps_t1")
                                nc.tensor.transpose(
                                    pst, x0[:, i, j * 128:(j + 1) * 128],
                                    ident32)
                                nc.vector.tensor_copy(
                                    xT0[:, j, i * 128:(i + 1) * 128], pst)
                        for et in range(8):
                            psq = psh.tile([128, 256], f32, tag="ps_q",
                                           name="ps_q")
                            for dt_ in range(DT):
                                nc.tensor.matmul(
                                    psq,
                                    lhsT=wqkv_sb[0][:, dt_,
                                                    et * 128:(et + 1) * 128],
                                    rhs=xT0[:, dt_, csl],
                                    start=(dt_ == 0), stop=(dt_ == DT - 1))
                            dst = (qT0[:, et, csl] if et < 4
                                   else kT0[:, et - 4, csl])
                            if has_qkv_b:
                                nc.scalar.activation(
                                    out=dst, in_=psq, func=AF.Identity,
                                    bias=bqkv_sb[0][:, et:et + 1], scale=1.0)
                            else:
                                nc.scalar.copy(out=dst, in_=psq)
                        # layer-0 V for these chunk tiles, reusing the freed
                        # segment-sum PSUM banks (their groups closed above).
                        for ct in cts:
                            psv = pseg.tile([128, D], f32, tag=f"sums{ct}",
                                            name=f"ps_v{ct}")
                            for dt_ in range(DT):
                                nc.tensor.matmul(
                                    psv,
                                    lhsT=xT0[:, dt_, ct * 128:(ct + 1) * 128],
                                    rhs=wqkv_sb[0][:, dt_, 2 * D:3 * D],
                                    start=(dt_ == 0), stop=(dt_ == DT - 1))
                            if has_qkv_b:
                                tv = lnp.tile([128, D], f32, tag="ln_t",
                                              name="tv0")
                                nc.vector.tensor_tensor(out=tv, in0=psv,
                                                        in1=vb_sb[0],
                                                        op=AL.add)
                                nc.scalar.copy(out=v_ext0[:, ct, :, 0:HD],
                                               in_=tv)
                            else:
                                nc.scalar.copy(out=v_ext0[:, ct, :, 0:HD],
                                               in_=psv)

                    hoist_t = last_t[1] if last_t[1] < NT - 1 else None
                    # tokens stream in groups of 4 tiles per dma_start —
                    # HWDGE issue costs ~625 ns per instruction, so 64
                    # single-tile loads would serialize ~40 us of issue.
                    for tg in range(NT // 4):
                        if tg in tok_gates:
                            tok4 = tok_gates[tg]
                        else:
                            tok4 = toks.tile([128, 4, D], bf16, tag="tok",
                                             name="tok4")
                        nc.scalar.dma_start(
                            out=tok4,
                            in_=tokens[tg * 512:(tg + 1) * 512, :].rearrange(
                                "(tt p) d -> p tt d", p=128))
                        for t2 in range(4):
                            t = tg * 4 + t2
                            lo, hi = ranges[t]
                            oh = ohs.tile([128, C], bf16, tag="oh",
                                           name="oh")
                            sl = slice(lo * 128, (hi + 1) * 128)
                            nc.vector.tensor_scalar(
                                out=oh[:, sl], in0=iota_row_sb[:, sl],
                                scalar1=seg_col_sb[:, t:t + 1],
                                scalar2=None, op0=AL.is_equal)
                            for m in range(lo, hi + 1):
                                nc.tensor.matmul(
                                    ps_sums[m],
                                    lhsT=oh[:, m * 128:(m + 1) * 128],
                                    rhs=tok4[:, t2, :],
                                    start=(t == first_t[m]),
                                    stop=(t == last_t[m]))
                            if t == hoist_t:
                                hoist_half(0)
                    if hoist_t is None:
                        hoist_half(0)
                    hoist_half(1)
                    load_late_weights()

                # ---------------- phase 2: transformer ----------------
                with (
                    tc.tile_pool(name="psA", bufs=2, space="PSUM") as psA,
                    tc.tile_pool(name="psS", bufs=2, space="PSUM") as psS,
                    tc.tile_pool(name="psO", bufs=2, space="PSUM") as psO,
                ):
                    def transpose_to(src_f32, dst_bf16):
                        # src: [128, CT, D] f32 token-major; dst: [128, DT, C] bf16
                        for i in range(CT):
                            for j in range(DT):
                                pst = psA.tile([128, 128], f32, tag="ps_t", name="ps_t")
                                nc.tensor.transpose(
                                    pst, src_f32[:, i, j * 128:(j + 1) * 128], ident32)
                                nc.vector.tensor_copy(
                                    dst_bf16[:, j, i * 128:(i + 1) * 128], pst)

                    x_in = x0
                    for l in range(L):
                        if l == 0:
                            # built during phase 1 (hoisted)
                            xT, qT, kT = xT0, qT0, kT0
                        else:
                            xT = xtp.tile([128, DT, C], bf16, tag="xT",
                                          name="xT")
                            transpose_to(x_in, xT)

                            # --- q, k feature-major [e, c] ---
                            qT = acts.tile([128, DT, C], bf16, tag="qT",
                                           name="qT")
                            kT = acts.tile([128, DT, C], bf16, tag="kT",
                                           name="kT")
                            for et in range(8):
                                ps = psA.tile([128, C], f32, tag="ps_a",
                                              name="ps_a")
                                for dt_ in range(DT):
                                    nc.tensor.matmul(
                                        ps,
                                        lhsT=wqkv_sb[l][:, dt_,
                                                        et * 128:(et + 1) * 128],
                                        rhs=xT[:, dt_, :],
                                        start=(dt_ == 0), stop=(dt_ == DT - 1))
                                dst = (qT[:, et, :] if et < 4
                                       else kT[:, et - 4, :])
                                if has_qkv_b:
                                    nc.scalar.activation(
                                        out=dst, in_=ps, func=AF.Identity,
                                        bias=bqkv_sb[l][:, et:et + 1], scale=1.0)
                                else:
                                    nc.scalar.copy(out=dst, in_=ps)

                        # --- v token-major [c, e] with per-head ones column ---
                        if l == 0:
                            v_ext = v_ext0
                        else:
                            v_ext = acts.tile([128, CT, H, HD + 1], bf16,
                                              tag="v_ext", name="v_ext")
                            nc.vector.memset(v_ext[:, :, :, HD:HD + 1], 1.0)
                        for ct in (() if l == 0 else range(CT)):
                            ps = psA.tile([128, C], f32, tag="ps_a", name="ps_a")
                            for dt_ in range(DT):
                                nc.tensor.matmul(
                                    ps, lhsT=xT[:, dt_, ct * 128:(ct + 1) * 128],
                                    rhs=wqkv_sb[l][:, dt_, 2 * D:3 * D],
                                    start=(dt_ == 0), stop=(dt_ == DT - 1))
                            if has_qkv_b:
                                tv = lnp.tile([128, D], f32, tag="ln_t", name="tv")
                                nc.vector.tensor_tensor(out=tv, in0=ps, in1=vb_sb[l],
                                                        op=AL.add)
                                nc.scalar.copy(out=v_ext[:, ct, :, 0:HD], in_=tv)
                            else:
                                nc.scalar.copy(out=v_ext[:, ct, :, 0:HD], in_=ps)

                        # --- attention: scores k-major; AV swapped so the
                        # output lands q-on-partitions — the softmax
                        # denominator (ones column of v_ext) becomes a
                        # per-partition column, so reciprocal + normalize are
                        # cheap per-partition DVE ops (no 1-partition recip,
                        # no fp32 broadcast matmul). Head pairs then transpose
                        # back to feature-major for the out-projection.
                        oT = acts.tile([128, DT, C], bf16, tag="oT", name="oT")

                        def score_exp_one(h, expT, kt):
                            th, off = h // 2, (h % 2) * 64
                            ps = psS.tile([128, C], f32, tag="ps_s",
                                          name="ps_s")
                            nc.tensor.matmul(
                                ps,
                                lhsT=kT[off:off + 64, th,
                                        kt * 128:(kt + 1) * 128],
                                rhs=qT[off:off + 64, th, :],
                                start=True, stop=True)
                            nc.scalar.activation(out=expT[:, kt, :], in_=ps,
                                                 func=AF.Exp, scale=1.0 / 8.0)

                        def av_norm(h, expT, opair):
                            off = (h % 2) * 64
                            for qt in range(CT):
                                pso = psO.tile([128, HD + 1], f32, tag="ps_o",
                                               name="ps_o")
                                for kt in range(CT):
                                    nc.tensor.matmul(
                                        pso,
                                        lhsT=expT[:, kt, qt * 128:(qt + 1) * 128],
                                        rhs=v_ext[:, kt, h, :],
                                        start=(kt == 0), stop=(kt == CT - 1))
                                rs = lnp.tile([128, 1], f32, tag="rs_att",
                                              name="rs_att")
                                nc.vector.reciprocal(rs, pso[:, HD:HD + 1])
                                nc.vector.tensor_scalar(
                                    out=opair[:, qt, off:off + 64],
                                    in0=pso[:, 0:HD], scalar1=rs[:, 0:1],
                                    scalar2=None, op0=AL.mult)

                        def pair_transpose(P, opair):
                            for qt in range(CT):
                                pst = psA.tile([128, 128], f32, tag="ps_t",
                                               name="ps_t")
                                nc.tensor.transpose(pst, opair[:, qt, :],
                                                    ident32)
                                nc.vector.tensor_copy(
                                    oT[:, P, qt * 128:(qt + 1) * 128], pst)

                        # Heads in groups of 4: a burst of row-tiled score
                        # matmuls (even/odd head pairs concurrent), then the
                        # AV + normalize + pair-transpose block in full
                        # 128x128 mode — minimizes PE tiling-mode switches.
                        for grp in range(H // 4):
                            heads = range(grp * 4, grp * 4 + 4)
                            expTs, opairs = {}, {}
                            for h in heads:
                                expTs[h] = expq.tile([128, CT, C], bf16,
                                                     tag="expT", name="expT")
                                if h % 2 == 0:
                                    opairs[h // 2] = opp.tile(
                                        [128, CT, 128], f32, tag="opair",
                                        name="opair")
                            for kt in range(CT):
                                for h in heads:
                                    score_exp_one(h, expTs[h], kt)
                            for h in heads:
                                av_norm(h, expTs[h], opairs[h // 2])
                                if h % 2 == 1:
                                    pair_transpose(h // 2, opairs[h // 2])

                        # --- out-projection + residual + LN1 ---
                        xm2 = xmp.tile([128, CT, D], f32, tag="xm", name="xm2")
                        for ct in range(CT):
                            ps = psA.tile([128, C], f32, tag="ps_a", name="ps_a")
                            for et in range(DT):
                                nc.tensor.matmul(
                                    ps, lhsT=oT[:, et, ct * 128:(ct + 1) * 128],
                                    rhs=wo_sb[l][:, et, :],
                                    start=(et == 0), stop=(et == DT - 1))
                            if has_out_b:
                                nc.vector.tensor_tensor(out=ps, in0=ps, in1=outb_sb[l],
                                                        op=AL.add)
                            ln_block(ps, x_in[:, ct, :],
                                     ln1w_sb[l] if has_ln1 else None,
                                     ln1b_sb[l] if has_ln1 else None,
                                     xm2[:, ct, :])

                        # --- FFN ---
                        x2T = xtp.tile([128, DT, C], bf16, tag="xT", name="x2T")
                        transpose_to(xm2, x2T)
                        hT = acts.tile([128, FT, C], bf16, tag="hT", name="hT")
                        for ft in range(FT):
                            ps = psA.tile([128, C], f32, tag="ps_a", name="ps_a")
                            for dt_ in range(DT):
                                nc.tensor.matmul(
                                    ps, lhsT=w1_sb[l][:, dt_, ft * 128:(ft + 1) * 128],
                                    rhs=x2T[:, dt_, :],
                                    start=(dt_ == 0), stop=(dt_ == DT - 1))
                            nc.scalar.activation(
                                out=hT[:, ft, :], in_=ps, func=AF.Relu,
                                bias=(b1_sb[l][:, ft:ft + 1] if has_ff1_b else 0.0),
                                scale=1.0)
                        x_next = xmp.tile([128, CT, D], f32, tag="xm", name="x_next")
                        for ct in range(CT):
                            ps = psA.tile([128, C], f32, tag="ps_a", name="ps_a")
                            for ft in range(FT):
                                nc.tensor.matmul(
                                    ps, lhsT=hT[:, ft, ct * 128:(ct + 1) * 128],
                                    rhs=w2_sb[l][:, ft, :],
                                    start=(ft == 0), stop=(ft == FT - 1))
                            if has_ff2_b:
                                nc.vector.tensor_tensor(out=ps, in0=ps, in1=ff2b_sb[l],
                                                        op=AL.add)
                            ln_block(ps, xm2[:, ct, :],
                                     ln2w_sb[l] if has_ln2 else None,
                                     ln2b_sb[l] if has_ln2 else None,
                                     x_next[:, ct, :])
                        x_in = x_next

                    # ---------------- phase 3: final LN -> y_bf ----------------
                    for ct in range(CT):
                        ln_block(x_in[:, ct, :], None, flnw_sb, flnb_sb,
                                 y_bf[:, ct, :])

            # ============ scope: expand ============
            with (
                tc.tile_pool(name="ohp", bufs=3) as ohp,
                tc.tile_pool(name="outp", bufs=3) as outp,
                tc.tile_pool(name="psE", bufs=4, space="PSUM") as psE,
            ):
                seg_row_ap = seg_row[:, :]
                for g in range(6, NT // 4):
                    sb = ohp.tile([128, 512], f16, tag=f"seg_bc{g}",
                                  name=f"seg_bc{g}")
                    src = bass.AP(tensor=seg_row_ap.tensor, offset=g * 512,
                                  ap=[[0, 128], [1, 512]])
                    nc.gpsimd.dma_start(out=sb, in_=src)
                    seg_bcs.append(sb)
                for g in range(NT // 4):
                    seg_bc = seg_bcs[g]
                    g_lo = min(ranges[g * 4 + i][0] for i in range(4))
                    g_hi = max(ranges[g * 4 + i][1] for i in range(4))
                    ohT = ohp.tile([128, CT, 512], bf16, tag="ohT", name="ohT")
                    for m in range(g_lo, g_hi + 1):
                        nc.vector.tensor_scalar(
                            out=ohT[:, m, :], in0=seg_bc,
                            scalar1=iota_col_sb[:, m:m + 1], scalar2=None,
                            op0=AL.is_equal)
                    ot4 = outp.tile([128, 4, D], f32, tag="ot", name="ot4")
                    for t2 in range(4):
                        t = g * 4 + t2
                        lo, hi = ranges[t]
                        pse = psE.tile([128, D], f32, tag="ps_e", name="ps_e")
                        for m in range(lo, hi + 1):
                            nc.tensor.matmul(
                                pse, lhsT=ohT[:, m, t2 * 128:(t2 + 1) * 128],
                                rhs=y_bf[:, m, :],
                                start=(m == lo), stop=(m == hi))
                        if t % 3 != 0:
                            nc.scalar.copy(out=ot4[:, t2, :], in_=pse)
                        else:
                            nc.vector.tensor_copy(ot4[:, t2, :], pse)
                    nc.sync.dma_start(
                        out=out_d[g * 512:(g + 1) * 512, :].rearrange(
                            "(tt p) d -> p tt d", p=128),
                        in_=ot4)

    return nc


def _host_prep(inputs):
    """Shard + preprocess full inputs into 8 per-core input maps."""
    tokens = np.asarray(inputs["tokens"], dtype=np.float32)
    seg = np.asarray(inputs["segment_ids"], dtype=np.int32)
    qkv_w = np.asarray(inputs["qkv_w"], dtype=np.float32)
    qkv_b = np.asarray(inputs["qkv_b"], dtype=np.float32)
    out_w = np.asarray(inputs["out_w"], dtype=np.float32)
    out_b = np.asarray(inputs["out_b"], dtype=np.float32)
    ln1_w = np.asarray(inputs["ln1_w"], dtype=np.float32)
    ln1_b = np.asarray(inputs["ln1_b"], dtype=np.float32)
    ln2_w = np.asarray(inputs["ln2_w"], dtype=np.float32)
    ln2_b = np.asarray(inputs["ln2_b"], dtype=np.float32)
    ff1_w = np.asarray(inputs["ff1_w"], dtype=np.float32)
    ff1_b = np.asarray(inputs["ff1_b"], dtype=np.float32)
    ff2_w = np.asarray(inputs["ff2_w"], dtype=np.float32)
    ff2_b = np.asarray(inputs["ff2_b"], dtype=np.float32)
    fln_w = np.asarray(inputs["fln_w"], dtype=np.float32)
    fln_b = np.asarray(inputs["fln_b"], dtype=np.float32)

    flags = (
        bool(np.any(qkv_b)),
        bool(np.any(out_b)),
        bool(np.any(ff1_b)),
        bool(np.any(ff2_b)),
        bool(np.any(ln1_w != 1.0) or np.any(ln1_b)),
        bool(np.any(ln2_w != 1.0) or np.any(ln2_b)),
        bool(np.any(fln_w != 1.0) or np.any(fln_b)),
    )

    # span-bound ranges: per token tile, union over batch of the contiguous
    # chunk-tile range its (sorted) segment ids cover.
    srt = np.all(np.diff(seg, axis=1) >= 0)
    if srt:
        lo = np.min(seg[:, ::128] // 128, axis=0)
        hi = np.max(seg[:, 127::128] // 128, axis=0)
    else:  # fallback: no structure assumed
        lo = np.zeros(NT, np.int64)
        hi = np.full(NT, CT - 1, np.int64)
    covered = set()
    for t in range(NT):
        covered.update(range(int(lo[t]), int(hi[t]) + 1))
    if covered != set(range(CT)):
        lo = np.zeros(NT, np.int64)
        hi = np.full(NT, CT - 1, np.int64)
    ranges = tuple((int(lo[t]), int(hi[t])) for t in range(NT))

    import ml_dtypes
    bf16 = ml_dtypes.bfloat16

    # shared (batch-independent) arrays
    shared = {
        "iota_row": np.broadcast_to(
            np.arange(C, dtype=np.float16)[None, :], (128, C)).copy(),
        "iota_col": (np.arange(CT, dtype=np.float32)[None, :] * 128
                     + np.arange(128, dtype=np.float32)[:, None]).astype(np.float32),
        "wqkvT": np.ascontiguousarray(qkv_w.transpose(0, 2, 1)).astype(bf16),
        "woT": np.ascontiguousarray(out_w.transpose(0, 2, 1)).astype(bf16),
        "w1T": np.ascontiguousarray(ff1_w.transpose(0, 2, 1)).astype(bf16),
        "w2T": np.ascontiguousarray(ff2_w.transpose(0, 2, 1)).astype(bf16),
    }
    (has_qkv_b, has_out_b, has_ff1_b, has_ff2_b,
     has_ln1, has_ln2, has_fln) = flags
    if has_qkv_b:
        shared["bqkv_c"] = np.ascontiguousarray(
            qkv_b[:, :1536].reshape(L, 12, 128).transpose(0, 2, 1))
        shared["vb_row"] = np.ascontiguousarray(qkv_b[:, 2 * D:3 * D][:, None, :])
    if has_ff1_b:
        shared["b1_c"] = np.ascontiguousarray(
            ff1_b.reshape(L, FT, 128).transpose(0, 2, 1))
    if has_out_b:
        shared["outb_row"] = np.ascontiguousarray(out_b[:, None, :])
    if has_ff2_b:
        shared["ff2b_row"] = np.ascontiguousarray(ff2_b[:, None, :])
    if has_ln1:
        shared["ln1w_row"] = np.ascontiguousarray(ln1_w[:, None, :])
        shared["ln1b_row"] = np.ascontiguousarray(ln1_b[:, None, :])
    if has_ln2:
        shared["ln2w_row"] = np.ascontiguousarray(ln2_w[:, None, :])
        shared["ln2b_row"] = np.ascontiguousarray(ln2_b[:, None, :])
    if has_fln:
        shared["flnw_row"] = np.ascontiguousarray(fln_w[None, :])
        shared["flnb_row"] = np.ascontiguousarray(fln_b[None, :])

    in_maps = []
    for b in range(B):
        m = dict(shared)
        m["tokens"] = np.ascontiguousarray(tokens[b]).astype(bf16)
        m["seg_col"] = np.ascontiguousarray(
            seg[b].reshape(NT, 128).T.astype(np.float32))
        m["seg_row"] = np.ascontiguousarray(seg[b].astype(np.float16)[None, :])
        cnt = np.bincount(seg[b], minlength=C).astype(np.float32)
        m["rcnt"] = np.ascontiguousarray((1.0 / cnt).reshape(CT, 128).T)
        in_maps.append(m)
    return flags, ranges, in_maps


def kernel(**inputs) -> np.ndarray:
    from concourse.bass_utils import run_bass_kernel_spmd

    flags, ranges, in_maps = _host_prep(inputs)
    key = (flags, ranges)
    if key not in _CACHE:
        nc = _build(flags, ranges)
        # bacc lowering (register alloc + multi-wait legalization) must run
        # before walrus codegen.
        if not nc.is_finalized():
            nc.finalize()
        _CACHE[key] = nc
    nc = _CACHE[key]
    res = run_bass_kernel_spmd(nc, in_maps, list(range(B)))
    return np.stack([res.results[i]["out"] for i in range(B)], axis=0)



# revision 52
# speedup vs baseline: 1.0147x; 1.0147x over previous
"""Trainium2 Bass kernel for nn_ChunkProcessor (segment-mean -> 2-layer
transformer encoder over chunks -> gather-expand -> final LN).

Sharding: data-parallel over batch B=8 across the 8 NeuronCores; each core
processes one batch item end to end (no cross-core communication).

Device pipeline per core:
  1. segment mean: one-hot(seg) matmuls on PE accumulate sums[C,D] and
     counts[C] in PSUM while token tiles stream in over HWDGE (cast to bf16
     on ACT/DVE). Because segment ids are sorted, each 128-token tile only
     touches a small contiguous range of chunk tiles; the SPMD program uses
     the union of those ranges across the 8 batch items (host-computed).
  2. transformer (bf16 matmuls, fp32 PSUM/LN): activations kept feature-major
     [d, c] for matmuls, token-major [c, d] for LayerNorm; softmax without
     max-subtraction (scores bounded ~1) so attention needs no big transposes.
  3. final LN on the [C, D] chunk table, then expand back to [S, D] with
     one-hot-transposed matmuls (same span-bound trick), streaming fp32
     tiles straight to HBM.
"""

import numpy as np

B, S, D = 8, 8192, 512
C, H, L, DFF = 512, 8, 2, 2048
HD = D // H          # 64
NT = S // 128        # 64 token tiles
CT = C // 128        # 4 chunk tiles
DT = D // 128        # 4 feature tiles
FT = DFF // 128      # 16
EPS = 1e-5

_CACHE = {}


def _build(flags, ranges):
    """Build the Bass program.

    flags  = (qkv_b, out_b, ff1_b, ff2_b, ln1_aff, ln2_aff, fln_aff) bools.
    ranges = tuple of (lo_m, hi_m) per token tile t: the contiguous range of
             chunk tiles any batch item's tile-t segment ids fall into.
    """
    import concourse.bass as bass
    import concourse.tile as tile
    from concourse import bacc, mybir
    from concourse.masks import make_identity

    (has_qkv_b, has_out_b, has_ff1_b, has_ff2_b,
     has_ln1, has_ln2, has_fln) = flags

    # first/last contributing token tile per chunk tile (for PSUM start/stop)
    first_t = [min(t for t in range(NT) if ranges[t][0] <= m <= ranges[t][1])
               for m in range(CT)]
    last_t = [max(t for t in range(NT) if ranges[t][0] <= m <= ranges[t][1])
              for m in range(CT)]

    f32 = mybir.dt.float32
    bf16 = mybir.dt.bfloat16
    f16 = mybir.dt.float16
    AL = mybir.AluOpType
    AF = mybir.ActivationFunctionType

    # Bacc (not raw Bass): its finalize pass legalizes multi-semaphore waits
    # into nop carriers — walrus TPB structs allow only ONE wait/instruction.
    nc = bacc.Bacc("TRN2", target_bir_lowering=False)

    tokens = nc.declare_dram_parameter("tokens", [S, D], bf16, isOutput=False)
    seg_col = nc.declare_dram_parameter("seg_col", [128, NT], f32, isOutput=False)
    seg_row = nc.declare_dram_parameter("seg_row", [1, S], f16, isOutput=False)
    iota_row = nc.declare_dram_parameter("iota_row", [128, C], f16, isOutput=False)
    iota_col = nc.declare_dram_parameter("iota_col", [128, CT], f32, isOutput=False)
    rcnt = nc.declare_dram_parameter("rcnt", [128, CT], f32, isOutput=False)
    wqkvT = nc.declare_dram_parameter("wqkvT", [L, D, 3 * D], bf16, isOutput=False)
    woT = nc.declare_dram_parameter("woT", [L, D, D], bf16, isOutput=False)
    w1T = nc.declare_dram_parameter("w1T", [L, D, DFF], bf16, isOutput=False)
    w2T = nc.declare_dram_parameter("w2T", [L, DFF, D], bf16, isOutput=False)
    if has_qkv_b:
        bqkv_c = nc.declare_dram_parameter("bqkv_c", [L, 128, 12], f32, isOutput=False)
        vb_row = nc.declare_dram_parameter("vb_row", [L, 1, D], f32, isOutput=False)
    if has_ff1_b:
        b1_c = nc.declare_dram_parameter("b1_c", [L, 128, FT], f32, isOutput=False)
    if has_out_b:
        outb_row = nc.declare_dram_parameter("outb_row", [L, 1, D], f32, isOutput=False)
    if has_ff2_b:
        ff2b_row = nc.declare_dram_parameter("ff2b_row", [L, 1, D], f32, isOutput=False)
    if has_ln1:
        ln1w_row = nc.declare_dram_parameter("ln1w_row", [L, 1, D], f32, isOutput=False)
        ln1b_row = nc.declare_dram_parameter("ln1b_row", [L, 1, D], f32, isOutput=False)
    if has_ln2:
        ln2w_row = nc.declare_dram_parameter("ln2w_row", [L, 1, D], f32, isOutput=False)
        ln2b_row = nc.declare_dram_parameter("ln2b_row", [L, 1, D], f32, isOutput=False)
    if has_fln:
        flnw_row = nc.declare_dram_parameter("flnw_row", [1, D], f32, isOutput=False)
        flnb_row = nc.declare_dram_parameter("flnb_row", [1, D], f32, isOutput=False)
    out_d = nc.declare_dram_parameter("out", [S, D], f32, isOutput=True)

    def bcast_load(pool, dram_row, tag):
        """DMA a [1, D] DRAM row into a [128, D] SBUF tile (partition bcast)."""
        t = pool.tile([128, D], f32, tag=tag, name=f"row_{tag}")
        src = bass.AP(tensor=dram_row.tensor, offset=dram_row.offset,
                      ap=[[0, 128]] + [list(p) for p in dram_row.ap[1:]])
        nc.gpsimd.dma_start(out=t, in_=src)
        return t

    with tile.TileContext(nc) as tc:
        with (
            tc.tile_pool(name="consts", bufs=1) as consts,
            tc.tile_pool(name="acts", bufs=1) as acts,
            tc.tile_pool(name="xm", bufs=2) as xmp,
            tc.tile_pool(name="xt", bufs=2) as xtp,
            tc.tile_pool(name="lnp", bufs=3) as lnp,
            tc.tile_pool(name="rows", bufs=1) as rows,
            tc.tile_pool(name="segbc", bufs=1) as segbc,
            tc.tile_pool(name="expq", bufs=4) as expq,
            tc.tile_pool(name="opp", bufs=2) as opp,
        ):
            # ---------------- constants ----------------
            seg_col_sb = consts.tile([128, NT], f32)
            nc.sync.dma_start(out=seg_col_sb, in_=seg_col[:, :])
            iota_row_sb = consts.tile([128, C], f16)
            nc.sync.dma_start(out=iota_row_sb, in_=iota_row[:, :])
            iota_col_sb = consts.tile([128, CT], f32)
            nc.sync.dma_start(out=iota_col_sb, in_=iota_col[:, :])
            rcnt_sb = consts.tile([128, CT], f32)
            nc.sync.dma_start(out=rcnt_sb, in_=rcnt[:, :])
            ident32 = consts.tile([128, 128], f32)
            make_identity(nc, ident32)
            ident_bf = consts.tile([128, 128], bf16)
            make_identity(nc, ident_bf)
            eps_t = consts.tile([128, 1], f32)
            nc.vector.memset(eps_t, EPS)

            # y_bf lives in the persistent pool (used by the expand phase
            # after the weight pool is closed).
            y_bf = acts.tile([128, CT, D], bf16, tag="y_bf")

            seg_bcs = []

            def ln_block(ps_src, resid, wrow, brow, out_ap):
                # out = LN(ps_src + resid) [* w + b]  (token-major)
                t_ = lnp.tile([128, D], f32, tag="ln_t", name="ln_t")
                if resid is not None:
                    nc.vector.tensor_tensor(out=t_, in0=ps_src, in1=resid, op=AL.add)
                else:
                    nc.vector.tensor_copy(t_, ps_src)
                st = lnp.tile([128, 6], f32, tag="ln_st", name="ln_st")
                nc.vector.bn_stats(out=st, in_=t_)
                mv = lnp.tile([128, 2], f32, tag="ln_mv", name="ln_mv")
                nc.vector.bn_aggr(out=mv, in_=st)
                sd = lnp.tile([128, 1], f32, tag="ln_sd", name="ln_sd")
                nc.scalar.activation(out=sd, in_=mv[:, 1:2], func=AF.Sqrt,
                                     bias=eps_t[:, 0:1], scale=1.0)
                rs = lnp.tile([128, 1], f32, tag="ln_rs", name="ln_rs")
                nc.vector.reciprocal(rs, sd)
                if wrow is None:
                    nc.vector.tensor_scalar(
                        out=out_ap, in0=t_, scalar1=mv[:, 0:1], scalar2=rs[:, 0:1],
                        op0=AL.subtract, op1=AL.mult)
                else:
                    xn = lnp.tile([128, D], f32, tag="ln_xn", name="ln_xn")
                    nc.vector.tensor_scalar(
                        out=xn, in0=t_, scalar1=mv[:, 0:1], scalar2=rs[:, 0:1],
                        op0=AL.subtract, op1=AL.mult)
                    nc.vector.tensor_tensor(out=xn, in0=xn, in1=wrow, op=AL.mult)
                    nc.vector.tensor_tensor(out=out_ap, in0=xn, in1=brow, op=AL.add)

            # ============ scope: weights + segsum + transformer ============
            with (
                tc.tile_pool(name="wts", bufs=1) as wts,
            ):
                # ---- weights (bf16, SWDGE) ----
                # Only layer 0's qkv/out weights load immediately; the rest
                # are gated behind the end of phase 1 (via a WAR dependency
                # on `gate`, which doubles as the last token tile's one-hot
                # buffer) so the token stream gets the HBM bandwidth first.
                wqkv_sb, wo_sb, w1_sb, w2_sb = [], [], [], []
                wq = wts.tile([128, DT, 3 * D], bf16, tag="wqkv0", name="wqkv0")
                nc.gpsimd.dma_start(
                    out=wq, in_=wqkvT[0].rearrange("(dt p) e -> p dt e", p=128))
                wqkv_sb.append(wq)
                wo = wts.tile([128, DT, D], bf16, tag="wo0", name="wo0")
                nc.gpsimd.dma_start(
                    out=wo, in_=woT[0].rearrange("(dt p) e -> p dt e", p=128))
                wo_sb.append(wo)

                # Late-weight tiles are allocated now but DMA'd only after the
                # last six token groups have landed IN these very buffers
                # (each tok4 group tg>=10 is staged in a slice of one late
                # tile) — the WAW/WAR dependency is the only thing the
                # scheduler respects, and it keeps these 8.6 MB of loads from
                # stealing HBM bandwidth from the token stream.
                w1_0 = wts.tile([128, DT, DFF], bf16, tag="w1l0", name="w1l0")
                w2_0 = wts.tile([128, FT, D], bf16, tag="w2l0", name="w2l0")
                wq_1 = wts.tile([128, DT, 3 * D], bf16, tag="wqkv1", name="wqkv1")
                wo_1 = wts.tile([128, DT, D], bf16, tag="wo1", name="wo1")
                w1_1 = wts.tile([128, DT, DFF], bf16, tag="w1l1", name="w1l1")
                w2_1 = wts.tile([128, FT, D], bf16, tag="w2l1", name="w2l1")
                w1_sb += [w1_0, w1_1]
                w2_sb += [w2_0, w2_1]
                wqkv_sb.append(wq_1)
                wo_sb.append(wo_1)
                tok_gates = {
                    10: w1_0[:, :, 0:512],
                    11: w2_0[:, 0:4, :],
                    12: wq_1[:, :, 0:512],
                    13: wo_1[:, :, :],
                    14: w1_1[:, :, 0:512],
                    15: w2_1[:, 0:4, :],
                }

                def load_late_weights():
                    nc.gpsimd.dma_start(
                        out=w1_0, in_=w1T[0].rearrange("(dt p) e -> p dt e", p=128))
                    nc.gpsimd.dma_start(
                        out=w2_0, in_=w2T[0].rearrange("(ft p) e -> p ft e", p=128))
                    nc.gpsimd.dma_start(
                        out=wq_1, in_=wqkvT[1].rearrange("(dt p) e -> p dt e", p=128))
                    nc.gpsimd.dma_start(
                        out=wo_1, in_=woT[1].rearrange("(dt p) e -> p dt e", p=128))
                    nc.gpsimd.dma_start(
                        out=w1_1, in_=w1T[1].rearrange("(dt p) e -> p dt e", p=128))
                    nc.gpsimd.dma_start(
                        out=w2_1, in_=w2T[1].rearrange("(ft p) e -> p ft e", p=128))
                    # prefetch the first few expand-phase segment-id
                    # broadcasts (tiny 1 KB reads, 128x SBUF fanout); the
                    # rest are loaded at the start of phase 3.
                    seg_row_ap = seg_row[:, :]
                    for g in range(6):
                        sb = segbc.tile([128, 512], f16, tag=f"seg_bc{g}",
                                        name=f"seg_bc{g}")
                        src = bass.AP(tensor=seg_row_ap.tensor, offset=g * 512,
                                      ap=[[0, 128], [1, 512]])
                        nc.gpsimd.dma_start(out=sb, in_=src)
                        seg_bcs.append(sb)

                bqkv_sb, b1_sb = [], []
                vb_sb, outb_sb, ff2b_sb = [], [], []
                ln1w_sb, ln1b_sb, ln2w_sb, ln2b_sb = [], [], [], []
                for l in range(L):
                    if has_qkv_b:
                        bq = consts.tile([128, 12], f32, tag=f"bqkv{l}", name=f"bqkv{l}")
                        nc.sync.dma_start(out=bq, in_=bqkv_c[l])
                        bqkv_sb.append(bq)
                        vb_sb.append(bcast_load(rows, vb_row[l], f"vb{l}"))
                    if has_ff1_b:
                        b1 = consts.tile([128, FT], f32, tag=f"b1{l}", name=f"b1{l}")
                        nc.sync.dma_start(out=b1, in_=b1_c[l])
                        b1_sb.append(b1)
                    if has_out_b:
                        outb_sb.append(bcast_load(rows, outb_row[l], f"outb{l}"))
                    if has_ff2_b:
                        ff2b_sb.append(bcast_load(rows, ff2b_row[l], f"ff2b{l}"))
                    if has_ln1:
                        ln1w_sb.append(bcast_load(rows, ln1w_row[l], f"ln1w{l}"))
                        ln1b_sb.append(bcast_load(rows, ln1b_row[l], f"ln1b{l}"))
                    if has_ln2:
                        ln2w_sb.append(bcast_load(rows, ln2w_row[l], f"ln2w{l}"))
                        ln2b_sb.append(bcast_load(rows, ln2b_row[l], f"ln2b{l}"))
                flnw_sb = bcast_load(rows, flnw_row, "flnw") if has_fln else None
                flnb_sb = bcast_load(rows, flnb_row, "flnb") if has_fln else None

                # ------------ phase 1: segment sums ------------
                # Layer-0 activations are built INSIDE phase 1 where data
                # permits: chunk tiles 0-1 close early (sorted segment ids),
                # so their divide + transpose + qkv-q/k matmuls are hoisted
                # into the PE idle time while the token tail streams in.
                x0 = xmp.tile([128, CT, D], f32, tag="xm", name="x0")
                xT0 = xtp.tile([128, DT, C], bf16, tag="xT", name="xT0")
                qT0 = acts.tile([128, DT, C], bf16, tag="qT", name="qT0")
                kT0 = acts.tile([128, DT, C], bf16, tag="kT", name="kT0")
                v_ext0 = acts.tile([128, CT, H, HD + 1], bf16, tag="v_ext",
                                   name="v_ext0")
                nc.vector.memset(v_ext0[:, :, :, HD:HD + 1], 1.0)
                with (
                    tc.tile_pool(name="pseg", bufs=1, space="PSUM") as pseg,
                    tc.tile_pool(name="psh", bufs=2, space="PSUM") as psh,
                    tc.tile_pool(name="toks", bufs=2) as toks,
                    tc.tile_pool(name="ohs", bufs=4) as ohs,
                ):
                    ps_sums = [pseg.tile([128, D], f32, tag=f"sums{m}", name=f"sums{m}")
                               for m in range(CT)]

                    def hoist_half(half):
                        cts = (0, 1) if half == 0 else (2, 3)
                        csl = slice(half * 256, half * 256 + 256)
                        for m in cts:
                            nc.vector.tensor_scalar(
                                out=x0[:, m, :], in0=ps_sums[m],
                                scalar1=rcnt_sb[:, m:m + 1],
                                scalar2=None, op0=AL.mult)
                        for i in cts:
                            for j in range(DT):
                                pst = psh.tile([128, 128], f32, tag="ps_t1",
                                               name="ps_t1")
                                nc.tensor.transpose(
                                    pst, x0[:, i, j * 128:(j + 1) * 128],
                                    ident32)
                                nc.vector.tensor_copy(
                                    xT0[:, j, i * 128:(i + 1) * 128], pst)
                        for et in range(8):
                            psq = psh.tile([128, 256], f32, tag="ps_q",
                                           name="ps_q")
                            for dt_ in range(DT):
                                nc.tensor.matmul(
                                    psq,
                                    lhsT=wqkv_sb[0][:, dt_,
                                                    et * 128:(et + 1) * 128],
                                    rhs=xT0[:, dt_, csl],
                                    start=(dt_ == 0), stop=(dt_ == DT - 1))
                            dst = (qT0[:, et, csl] if et < 4
                                   else kT0[:, et - 4, csl])
                            if has_qkv_b:
                                nc.scalar.activation(
                                    out=dst, in_=psq, func=AF.Identity,
                                    bias=bqkv_sb[0][:, et:et + 1], scale=1.0)
                            else:
                                nc.vector.tensor_copy(dst, psq)
                        # layer-0 V for these chunk tiles, reusing the freed
                        # segment-sum PSUM banks (their groups closed above).
                        for ct in cts:
                            psv = pseg.tile([128, D], f32, tag=f"sums{ct}",
                                            name=f"ps_v{ct}")
                            for dt_ in range(DT):
                                nc.tensor.matmul(
                                    psv,
                                    lhsT=xT0[:, dt_, ct * 128:(ct + 1) * 128],
                                    rhs=wqkv_sb[0][:, dt_, 2 * D:3 * D],
                                    start=(dt_ == 0), stop=(dt_ == DT - 1))
                            if has_qkv_b:
                                tv = lnp.tile([128, D], f32, tag="ln_t",
                                              name="tv0")
                                nc.vector.tensor_tensor(out=tv, in0=psv,
                                                        in1=vb_sb[0],
                                                        op=AL.add)
                                nc.scalar.copy(out=v_ext0[:, ct, :, 0:HD],
                                               in_=tv)
                            else:
                                nc.vector.tensor_copy(v_ext0[:, ct, :, 0:HD],
                                                      psv)

                    hoist_t = last_t[1] if last_t[1] < NT - 1 else None
                    # tokens stream in groups of 4 tiles per dma_start —
                    # HWDGE issue costs ~625 ns per instruction, so 64
                    # single-tile loads would serialize ~40 us of issue.
                    for tg in range(NT // 4):
                        if tg in tok_gates:
                            tok4 = tok_gates[tg]
                        else:
                            tok4 = toks.tile([128, 4, D], bf16, tag="tok",
                                             name="tok4")
                        nc.scalar.dma_start(
                            out=tok4,
                            in_=tokens[tg * 512:(tg + 1) * 512, :].rearrange(
                                "(tt p) d -> p tt d", p=128))
                        for t2 in range(4):
                            t = tg * 4 + t2
                            lo, hi = ranges[t]
                            oh = ohs.tile([128, C], bf16, tag="oh",
                                           name="oh")
                            sl = slice(lo * 128, (hi + 1) * 128)
                            nc.vector.tensor_scalar(
                                out=oh[:, sl], in0=iota_row_sb[:, sl],
                                scalar1=seg_col_sb[:, t:t + 1],
                                scalar2=None, op0=AL.is_equal)
                            for m in range(lo, hi + 1):
                                nc.tensor.matmul(
                                    ps_sums[m],
                                    lhsT=oh[:, m * 128:(m + 1) * 128],
                                    rhs=tok4[:, t2, :],
                                    start=(t == first_t[m]),
                                    stop=(t == last_t[m]))
                            if t == hoist_t:
                                hoist_half(0)
                    if hoist_t is None:
                        hoist_half(0)
                    hoist_half(1)
                    load_late_weights()

                # ---------------- phase 2: transformer ----------------
                with (
                    tc.tile_pool(name="psA", bufs=2, space="PSUM") as psA,
                    tc.tile_pool(name="psS", bufs=2, space="PSUM") as psS,
                    tc.tile_pool(name="psO", bufs=2, space="PSUM") as psO,
                ):
                    def transpose_to(src_f32, dst_bf16):
                        # src: [128, CT, D] f32 token-major; dst: [128, DT, C] bf16
                        for i in range(CT):
                            for j in range(DT):
                                pst = psA.tile([128, 128], f32, tag="ps_t", name="ps_t")
                                nc.tensor.transpose(
                                    pst, src_f32[:, i, j * 128:(j + 1) * 128], ident32)
                                nc.vector.tensor_copy(
                                    dst_bf16[:, j, i * 128:(i + 1) * 128], pst)

                    x_in = x0
                    for l in range(L):
                        if l == 0:
                            # built during phase 1 (hoisted)
                            xT, qT, kT = xT0, qT0, kT0
                        else:
                            xT = xtp.tile([128, DT, C], bf16, tag="xT",
                                          name="xT")
                            transpose_to(x_in, xT)

                            # --- q, k feature-major [e, c] ---
                            qT = acts.tile([128, DT, C], bf16, tag="qT",
                                           name="qT")
                            kT = acts.tile([128, DT, C], bf16, tag="kT",
                                           name="kT")
                            for et in range(8):
                                ps = psA.tile([128, C], f32, tag="ps_a",
                                              name="ps_a")
                                for dt_ in range(DT):
                                    nc.tensor.matmul(
                                        ps,
                                        lhsT=wqkv_sb[l][:, dt_,
                                                        et * 128:(et + 1) * 128],
                                        rhs=xT[:, dt_, :],
                                        start=(dt_ == 0), stop=(dt_ == DT - 1))
                                dst = (qT[:, et, :] if et < 4
                                       else kT[:, et - 4, :])
                                if has_qkv_b:
                                    nc.scalar.activation(
                                        out=dst, in_=ps, func=AF.Identity,
                                        bias=bqkv_sb[l][:, et:et + 1], scale=1.0)
                                else:
                                    nc.scalar.copy(out=dst, in_=ps)

                        # --- v token-major [c, e] with per-head ones column ---
                        if l == 0:
                            v_ext = v_ext0
                        else:
                            v_ext = acts.tile([128, CT, H, HD + 1], bf16,
                                              tag="v_ext", name="v_ext")
                            nc.vector.memset(v_ext[:, :, :, HD:HD + 1], 1.0)
                        for ct in (() if l == 0 else range(CT)):
                            ps = psA.tile([128, C], f32, tag="ps_a", name="ps_a")
                            for dt_ in range(DT):
                                nc.tensor.matmul(
                                    ps, lhsT=xT[:, dt_, ct * 128:(ct + 1) * 128],
                                    rhs=wqkv_sb[l][:, dt_, 2 * D:3 * D],
                                    start=(dt_ == 0), stop=(dt_ == DT - 1))
                            if has_qkv_b:
                                tv = lnp.tile([128, D], f32, tag="ln_t", name="tv")
                                nc.vector.tensor_tensor(out=tv, in0=ps, in1=vb_sb[l],
                                                        op=AL.add)
                                nc.scalar.copy(out=v_ext[:, ct, :, 0:HD], in_=tv)
                            else:
                                nc.scalar.copy(out=v_ext[:, ct, :, 0:HD], in_=ps)

                        # --- attention: scores k-major; AV swapped so the
                        # output lands q-on-partitions — the softmax
                        # denominator (ones column of v_ext) becomes a
                        # per-partition column, so reciprocal + normalize are
                        # cheap per-partition DVE ops (no 1-partition recip,
                        # no fp32 broadcast matmul). Head pairs then transpose
                        # back to feature-major for the out-projection.
                        oT = acts.tile([128, DT, C], bf16, tag="oT", name="oT")

                        def score_exp_one(h, expT, kt):
                            th, off = h // 2, (h % 2) * 64
                            ps = psS.tile([128, C], f32, tag="ps_s",
                                          name="ps_s")
                            nc.tensor.matmul(
                                ps,
                                lhsT=kT[off:off + 64, th,
                                        kt * 128:(kt + 1) * 128],
                                rhs=qT[off:off + 64, th, :],
                                start=True, stop=True)
                            nc.scalar.activation(out=expT[:, kt, :], in_=ps,
                                                 func=AF.Exp, scale=1.0 / 8.0)

                        def av_norm(h, expT, opair):
                            off = (h % 2) * 64
                            for qt in range(CT):
                                pso = psO.tile([128, HD + 1], f32, tag="ps_o",
                                               name="ps_o")
                                for kt in range(CT):
                                    nc.tensor.matmul(
                                        pso,
                                        lhsT=expT[:, kt, qt * 128:(qt + 1) * 128],
                                        rhs=v_ext[:, kt, h, :],
                                        start=(kt == 0), stop=(kt == CT - 1))
                                rs = lnp.tile([128, 1], f32, tag="rs_att",
                                              name="rs_att")
                                nc.vector.reciprocal(rs, pso[:, HD:HD + 1])
                                nc.vector.tensor_scalar(
                                    out=opair[:, qt, off:off + 64],
                                    in0=pso[:, 0:HD], scalar1=rs[:, 0:1],
                                    scalar2=None, op0=AL.mult)

                        def pair_transpose(P, opair):
                            for qt in range(CT):
                                pst = psA.tile([128, 128], f32, tag="ps_t",
                                               name="ps_t")
                                nc.tensor.transpose(pst, opair[:, qt, :],
                                                    ident32)
                                nc.vector.tensor_copy(
                                    oT[:, P, qt * 128:(qt + 1) * 128], pst)

                        # Heads in groups of 4: a burst of row-tiled score
                        # matmuls (even/odd head pairs concurrent), then the
                        # AV + normalize + pair-transpose block in full
                        # 128x128 mode — minimizes PE tiling-mode switches.
                        for grp in range(H // 4):
                            heads = range(grp * 4, grp * 4 + 4)
                            expTs, opairs = {}, {}
                            for h in heads:
                                expTs[h] = expq.tile([128, CT, C], bf16,
                                                     tag="expT", name="expT")
                                if h % 2 == 0:
                                    opairs[h // 2] = opp.tile(
                                        [128, CT, 128], f32, tag="opair",
                                        name="opair")
                            for kt in range(CT):
                                for h in heads:
                                    score_exp_one(h, expTs[h], kt)
                            for h in heads:
                                av_norm(h, expTs[h], opairs[h // 2])
                                if h % 2 == 1:
                                    pair_transpose(h // 2, opairs[h // 2])

                        # --- out-projection + residual + LN1 ---
                        xm2 = xmp.tile([128, CT, D], f32, tag="xm", name="xm2")
                        for ct in range(CT):
                            ps = psA.tile([128, C], f32, tag="ps_a", name="ps_a")
                            for et in range(DT):
                                nc.tensor.matmul(
                                    ps, lhsT=oT[:, et, ct * 128:(ct + 1) * 128],
                                    rhs=wo_sb[l][:, et, :],
                                    start=(et == 0), stop=(et == DT - 1))
                            if has_out_b:
                                nc.vector.tensor_tensor(out=ps, in0=ps, in1=outb_sb[l],
                                                        op=AL.add)
                            ln_block(ps, x_in[:, ct, :],
                                     ln1w_sb[l] if has_ln1 else None,
                                     ln1b_sb[l] if has_ln1 else None,
                                     xm2[:, ct, :])

                        # --- FFN ---
                        x2T = xtp.tile([128, DT, C], bf16, tag="xT", name="x2T")
                        transpose_to(xm2, x2T)
                        hT = acts.tile([128, FT, C], bf16, tag="hT", name="hT")
                        for ft in range(FT):
                            ps = psA.tile([128, C], f32, tag="ps_a", name="ps_a")
                            for dt_ in range(DT):
                                nc.tensor.matmul(
                                    ps, lhsT=w1_sb[l][:, dt_, ft * 128:(ft + 1) * 128],
                                    rhs=x2T[:, dt_, :],
                                    start=(dt_ == 0), stop=(dt_ == DT - 1))
                            nc.scalar.activation(
                                out=hT[:, ft, :], in_=ps, func=AF.Relu,
                                bias=(b1_sb[l][:, ft:ft + 1] if has_ff1_b else 0.0),
                                scale=1.0)
                        x_next = xmp.tile([128, CT, D], f32, tag="xm", name="x_next")
                        for ct in range(CT):
                            ps = psA.tile([128, C], f32, tag="ps_a", name="ps_a")
                            for ft in range(FT):
                                nc.tensor.matmul(
                                    ps, lhsT=hT[:, ft, ct * 128:(ct + 1) * 128],
                                    rhs=w2_sb[l][:, ft, :],
                                    start=(ft == 0), stop=(ft == FT - 1))
                            if has_ff2_b:
                                nc.vector.tensor_tensor(out=ps, in0=ps, in1=ff2b_sb[l],
                                                        op=AL.add)
                            ln_block(ps, xm2[:, ct, :],
                                     ln2w_sb[l] if has_ln2 else None,
                                     ln2b_sb[l] if has_ln2 else None,
                                     x_next[:, ct, :])
                        x_in = x_next

                    # ---------------- phase 3: final LN -> y_bf ----------------
                    for ct in range(CT):
                        ln_block(x_in[:, ct, :], None, flnw_sb, flnb_sb,
                                 y_bf[:, ct, :])

            # ============ scope: expand ============
            with (
                tc.tile_pool(name="ohp", bufs=3) as ohp,
                tc.tile_pool(name="outp", bufs=3) as outp,
                tc.tile_pool(name="psE", bufs=4, space="PSUM") as psE,
            ):
                seg_row_ap = seg_row[:, :]
                for g in range(6, NT // 4):
                    sb = ohp.tile([128, 512], f16, tag=f"seg_bc{g}",
                                  name=f"seg_bc{g}")
                    src = bass.AP(tensor=seg_row_ap.tensor, offset=g * 512,
                                  ap=[[0, 128], [1, 512]])
                    nc.gpsimd.dma_start(out=sb, in_=src)
                    seg_bcs.append(sb)
                for g in range(NT // 4):
                    seg_bc = seg_bcs[g]
                    g_lo = min(ranges[g * 4 + i][0] for i in range(4))
                    g_hi = max(ranges[g * 4 + i][1] for i in range(4))
                    ohT = ohp.tile([128, CT, 512], bf16, tag="ohT", name="ohT")
                    for m in range(g_lo, g_hi + 1):
                        nc.vector.tensor_scalar(
                            out=ohT[:, m, :], in0=seg_bc,
                            scalar1=iota_col_sb[:, m:m + 1], scalar2=None,
                            op0=AL.is_equal)
                    ot4 = outp.tile([128, 4, D], f32, tag="ot", name="ot4")
                    for t2 in range(4):
                        t = g * 4 + t2
                        lo, hi = ranges[t]
                        pse = psE.tile([128, D], f32, tag="ps_e", name="ps_e")
                        for m in range(lo, hi + 1):
                            nc.tensor.matmul(
                                pse, lhsT=ohT[:, m, t2 * 128:(t2 + 1) * 128],
                                rhs=y_bf[:, m, :],
                                start=(m == lo), stop=(m == hi))
                        if t % 3 != 0:
                            nc.scalar.copy(out=ot4[:, t2, :], in_=pse)
                        else:
                            nc.vector.tensor_copy(ot4[:, t2, :], pse)
                    nc.sync.dma_start(
                        out=out_d[g * 512:(g + 1) * 512, :].rearrange(
                            "(tt p) d -> p tt d", p=128),
                        in_=ot4)

    return nc


def _host_prep(inputs):
    """Shard + preprocess full inputs into 8 per-core input maps."""
    tokens = np.asarray(inputs["tokens"], dtype=np.float32)
    seg = np.asarray(inputs["segment_ids"], dtype=np.int32)
    qkv_w = np.asarray(inputs["qkv_w"], dtype=np.float32)
    qkv_b = np.asarray(inputs["qkv_b"], dtype=np.float32)
    out_w = np.asarray(inputs["out_w"], dtype=np.float32)
    out_b = np.asarray(inputs["out_b"], dtype=np.float32)
    ln1_w = np.asarray(inputs["ln1_w"], dtype=np.float32)
    ln1_b = np.asarray(inputs["ln1_b"], dtype=np.float32)
    ln2_w = np.asarray(inputs["ln2_w"], dtype=np.float32)
    ln2_b = np.asarray(inputs["ln2_b"], dtype=np.float32)
    ff1_w = np.asarray(inputs["ff1_w"], dtype=np.float32)
    ff1_b = np.asarray(inputs["ff1_b"], dtype=np.float32)
    ff2_w = np.asarray(inputs["ff2_w"], dtype=np.float32)
    ff2_b = np.asarray(inputs["ff2_b"], dtype=np.float32)
    fln_w = np.asarray(inputs["fln_w"], dtype=np.float32)
    fln_b = np.asarray(inputs["fln_b"], dtype=np.float32)

    flags = (
        bool(np.any(qkv_b)),
        bool(np.any(out_b)),
        bool(np.any(ff1_b)),
        bool(np.any(ff2_b)),
        bool(np.any(ln1_w != 1.0) or np.any(ln1_b)),
        bool(np.any(ln2_w != 1.0) or np.any(ln2_b)),
        bool(np.any(fln_w != 1.0) or np.any(fln_b)),
    )

    # span-bound ranges: per token tile, union over batch of the contiguous
    # chunk-tile range its (sorted) segment ids cover.
    srt = np.all(np.diff(seg, axis=1) >= 0)
    if srt:
        lo = np.min(seg[:, ::128] // 128, axis=0)
        hi = np.max(seg[:, 127::128] // 128, axis=0)
    else:  # fallback: no structure assumed
        lo = np.zeros(NT, np.int64)
        hi = np.full(NT, CT - 1, np.int64)
    covered = set()
    for t in range(NT):
        covered.update(range(int(lo[t]), int(hi[t]) + 1))
    if covered != set(range(CT)):
        lo = np.zeros(NT, np.int64)
        hi = np.full(NT, CT - 1, np.int64)
    ranges = tuple((int(lo[t]), int(hi[t])) for t in range(NT))

    import ml_dtypes
    bf16 = ml_dtypes.bfloat16

    # shared (batch-independent) arrays
    shared = {
        "iota_row": np.broadcast_to(
            np.arange(C, dtype=np.float16)[None, :], (128, C)).copy(),
        "iota_col": (np.arange(CT, dtype=np.float32)[None, :] * 128
                     + np.arange(128, dtype=np.float32)[:, None]).astype(np.float32),
        "wqkvT": np.ascontiguousarray(qkv_w.transpose(0, 2, 1)).astype(bf16),
        "woT": np.ascontiguousarray(out_w.transpose(0, 2, 1)).astype(bf16),
        "w1T": np.ascontiguousarray(ff1_w.transpose(0, 2, 1)).astype(bf16),
        "w2T": np.ascontiguousarray(ff2_w.transpose(0, 2, 1)).astype(bf16),
    }
    (has_qkv_b, has_out_b, has_ff1_b, has_ff2_b,
     has_ln1, has_ln2, has_fln) = flags
    if has_qkv_b:
        shared["bqkv_c"] = np.ascontiguousarray(
            qkv_b[:, :1536].reshape(L, 12, 128).transpose(0, 2, 1))
        shared["vb_row"] = np.ascontiguousarray(qkv_b[:, 2 * D:3 * D][:, None, :])
    if has_ff1_b:
        shared["b1_c"] = np.ascontiguousarray(
            ff1_b.reshape(L, FT, 128).transpose(0, 2, 1))
    if has_out_b:
        shared["outb_row"] = np.ascontiguousarray(out_b[:, None, :])
    if has_ff2_b:
        shared["ff2b_row"] = np.ascontiguousarray(ff2_b[:, None, :])
    if has_ln1:
        shared["ln1w_row"] = np.ascontiguousarray(ln1_w[:, None, :])
        shared["ln1b_row"] = np.ascontiguousarray(ln1_b[:, None, :])
    if has_ln2:
        shared["ln2w_row"] = np.ascontiguousarray(ln2_w[:, None, :])
        shared["ln2b_row"] = np.ascontiguousarray(ln2_b[:, None, :])
    if has_fln:
        shared["flnw_row"] = np.ascontiguousarray(fln_w[None, :])
        shared["flnb_row"] = np.ascontiguousarray(fln_b[None, :])

    in_maps = []
    for b in range(B):
        m = dict(shared)
        m["tokens"] = np.ascontiguousarray(tokens[b]).astype(bf16)
        m["seg_col"] = np.ascontiguousarray(
            seg[b].reshape(NT, 128).T.astype(np.float32))
        m["seg_row"] = np.ascontiguousarray(seg[b].astype(np.float16)[None, :])
        cnt = np.bincount(seg[b], minlength=C).astype(np.float32)
        m["rcnt"] = np.ascontiguousarray((1.0 / cnt).reshape(CT, 128).T)
        in_maps.append(m)
    return flags, ranges, in_maps


def kernel(**inputs) -> np.ndarray:
    from concourse.bass_utils import run_bass_kernel_spmd

    flags, ranges, in_maps = _host_prep(inputs)
    key = (flags, ranges)
    if key not in _CACHE:
        nc = _build(flags, ranges)
        # bacc lowering (register alloc + multi-wait legalization) must run
        # before walrus codegen.
        if not nc.is_finalized():
            nc.finalize()
        _CACHE[key] = nc
    nc = _CACHE[key]
    res = run_bass_kernel_spmd(nc, in_maps, list(range(B)))
    return np.stack([res.results[i]["out"] for i in range(B)], axis=0)

